# revision 3
# baseline (speedup 1.0000x reference)
"""Trainium2 Bass kernel for nn_CNOLReLu: bicubic 2x upsample -> leaky_relu
-> antialiased bicubic 2x downsample on a (16,128,128,128) NHWC tensor.

Data-parallel over batch: 2 images per NeuronCore.  Per channel c the op is
Y = D @ f(U @ X @ U.T) @ D.T with X = x[b,:,:,c], U = 128->256 bicubic,
D = 256->128 antialiased bicubic, f = leaky_relu(0.01).  Four matmul hops,
processed in groups of 4 channels:
  A: pA[w,  (ci,h2)]   = X_c^T @ U^T        (data-stationary, per channel)
  B: pZ_t[w2t, (j,h2)] = U_t @ sP           (matrix-stationary, per w2-chunk)
  f: Lrelu on ScalarE during PSUM->SBUF evac (FD=1024 per instr)
  C: pS[h2m, m*512+j*128+w'] = sA_jmt^T @ D_t^T  (data-stationary, banded)
  D: pY[h', (j,w')]    = D_m^T-chunks @ sS  (matrix-stationary, N=512)
Engines: ScalarE does the two activations + D-evac; VectorE does A/C evacs.
"""
import numpy as np
import ml_dtypes
from contextlib import ExitStack

import concourse.bacc as bacc
import concourse.tile as tile
from concourse import mybir
from concourse.bass_utils import run_bass_kernel_spmd

F32 = mybir.dt.float32
BF16 = mybir.dt.bfloat16
AF = mybir.ActivationFunctionType

N_CORES = 8
B_CORE = 2          # images per core
H = W = C = 128
NEG_SLOPE = 0.01


def _keys_cubic(x):
    x = np.abs(x)
    return np.where(
        x <= 1, (1.5 * x - 2.5) * x * x + 1,
        np.where(x < 2, ((-0.5 * x + 2.5) * x - 4) * x + 2, 0.0))


def _resize_matrix(n_in, n_out):
    """Row-stochastic bicubic (antialias) resize operator; matches
    jax.image.resize(method='bicubic', antialias=True)."""
    scale = n_out / n_in
    pos = (np.arange(n_out) + 0.5) / scale - 0.5
    kscale = min(scale, 1.0)
    w = _keys_cubic((np.arange(n_in)[None, :] - pos[:, None]) * kscale)
    return (w / w.sum(axis=1, keepdims=True)).astype(np.float64)


def _band(Dm, t):
    rows = np.nonzero(np.abs(Dm[:, t * 128:(t + 1) * 128]).sum(1) > 0)[0]
    return int(rows.min()), int(rows.max()) + 1


_CACHE = {}


def _build():
    if "nc" in _CACHE:
        return _CACHE["nc"], _CACHE["consts"]

    U = _resize_matrix(H, 2 * H)       # [256,128]
    Dm = _resize_matrix(2 * H, H)      # [128,256]
    uT = U.T.astype(ml_dtypes.bfloat16)                              # [128,256]
    dT = np.concatenate([Dm.T[0:128, :], Dm.T[128:256, :]], axis=1)  # [128,256]
    dT_bf = dT.astype(ml_dtypes.bfloat16)
    bands = [_band(Dm, 0), _band(Dm, 1)]   # [(0,66),(62,128)]

    nc = bacc.Bacc()
    x_d = nc.declare_dram_parameter("x", [B_CORE, H, W, C], BF16, isOutput=False)
    ut_d = nc.declare_dram_parameter("ut", [128, 256], BF16, isOutput=False)
    dbf_d = nc.declare_dram_parameter("dbf", [128, 256], BF16, isOutput=False)
    y_d = nc.declare_dram_parameter("y", [B_CORE, H, W, C], BF16, isOutput=True)

    with tile.TileContext(nc) as tc, ExitStack() as ctx:
        wpool = ctx.enter_context(tc.tile_pool(name="weights", bufs=1))
        xpool = ctx.enter_context(tc.tile_pool(name="ximg", bufs=2))
        opool = ctx.enter_context(tc.tile_pool(name="oimg", bufs=2))
        spool = ctx.enter_context(tc.tile_pool(name="stage", bufs=2))
        ppool = ctx.enter_context(tc.tile_pool(name="psum", bufs=1, space="PSUM"))

        ut_s = wpool.tile([128, 256], BF16, tag="ut")
        dbf_s = wpool.tile([128, 256], BF16, tag="dbf")
        nc.sync.dma_start(ut_s[:], ut_d[:])
        nc.sync.dma_start(dbf_s[:], dbf_d[:])

        for b in range(B_CORE):
            ximg = xpool.tile([128, W * C], BF16, tag="ximg")
            nc.sync.dma_start(ximg[:], x_d[b].rearrange("h w c -> h (w c)"))
            oimg = opool.tile([128, W * C], BF16, tag="oimg")

            for g in range(C // 4):          # 4-channel groups
                c0 = g * 4
                # ---- A: per pair p, pA_p[w, (ci,h2)] = X_c^T @ U^T
                sP = spool.tile([128, 1024], BF16, tag="sP")
                for p in range(2):
                    pA = ppool.tile([128, 512], F32, tag="pA", bufs=2)
                    for ci in range(2):
                        nc.tensor.matmul(pA[:, ci * 256:(ci + 1) * 256],
                                         ximg[:, (c0 + 2 * p + ci)::C], ut_s[:],
                                         start=True, stop=True)
                    nc.vector.tensor_copy(sP[:, p * 512:(p + 1) * 512], pA[:])

                # ---- B: pZ_t[w2t, (j,h2)] = U_t @ sP ; lrelu -> sA
                sA = spool.tile([128, 2048], BF16, tag="sA")
                for t in range(2):
                    pZ = ppool.tile([128, 1024], F32, tag="pZ", bufs=1)
                    for p in range(2):
                        nc.tensor.matmul(pZ[:, p * 512:(p + 1) * 512],
                                         ut_s[:, t * 128:(t + 1) * 128],
                                         sP[:, p * 512:(p + 1) * 512],
                                         start=True, stop=True)
                    nc.scalar.activation(sA[:, t * 1024:(t + 1) * 1024],
                                         pZ[:], AF.Lrelu, alpha=NEG_SLOPE)

                # ---- C: banded W-down; pS cols = m*512 + j*128 + w'
                pS = ppool.tile([128, 1024], F32, tag="pS", bufs=1)
                for j in range(4):
                    for m in range(2):
                        for t in range(2):
                            lo, hi = bands[t]
                            nc.tensor.matmul(
                                pS[:, m * 512 + j * 128 + lo:
                                   m * 512 + j * 128 + hi],
                                sA[:, t * 1024 + j * 256 + m * 128:
                                   t * 1024 + j * 256 + (m + 1) * 128],
                                dbf_s[:, t * 128 + lo:t * 128 + hi],
                                start=(t == 0), stop=(t == 1),
                                skip_group_check=True)
                sS = spool.tile([128, 1024], BF16, tag="sS")
                nc.vector.tensor_copy(sS[:], pS[:])

                # ---- D: pY[h', (j,w')] = sum_m D-chunk_m @ sS[:, m*512:]
                pY = ppool.tile([128, 512], F32, tag="pY", bufs=2)
                for m in range(2):
                    nc.tensor.matmul(pY[:],
                                     dbf_s[:, m * 128:(m + 1) * 128],
                                     sS[:, m * 512:(m + 1) * 512],
                                     start=(m == 0), stop=(m == 1))
                # ---- evac pY (j,w') -> oimg cols w'*C + c0 + j
                dsto = oimg[:].rearrange(
                    "h (w c) -> h w c", c=C)[:, :, c0:c0 + 4]
                srco = pY[:].rearrange("h (c w) -> h w c", c=4)
                nc.scalar.copy(dsto, srco)

            nc.sync.dma_start(y_d[b].rearrange("h w c -> h (w c)"), oimg[:])

    nc.compile()
    consts = {"ut": np.ascontiguousarray(uT),
              "dbf": np.ascontiguousarray(dT_bf)}
    _CACHE["nc"] = nc
    _CACHE["consts"] = consts
    return nc, consts


def prepare(x):
    x = np.asarray(x, dtype=np.float32)
    assert x.shape == (16, H, W, C), x.shape
    nc, consts = _build()
    in_maps = []
    for core in range(N_CORES):
        m = {"x": np.ascontiguousarray(
            x[core * B_CORE:(core + 1) * B_CORE]).astype(ml_dtypes.bfloat16)}
        m.update(consts)
        in_maps.append(m)
    return nc, in_maps


def kernel(x, in_size=128, out_size=128, trace=False, tmpdir=None):
    nc, in_maps = prepare(x)
    res = run_bass_kernel_spmd(nc, in_maps, list(range(N_CORES)), trace=trace,
                               tmpdir=tmpdir)
    out = np.concatenate([res.results[i]["y"] for i in range(N_CORES)], axis=0)
    if trace:
        kernel.last_exec_time_ns = res.exec_time_ns
        kernel.last_results = res
    return out.astype(np.float32)


# revision 6
# speedup vs baseline: 7.7987x; 7.7987x over previous
"""Trainium2 Bass kernel for nn_CNOLReLu: bicubic 2x upsample -> leaky_relu
-> antialiased bicubic 2x downsample on a (16,128,128,128) NHWC tensor.

Data-parallel over batch: 2 images per NeuronCore.  Per channel c the op is
Y = D @ f(U @ X @ U.T) @ D.T with X = x[b,:,:,c], U = 128->256 bicubic,
D = 256->128 antialiased bicubic, f = leaky_relu(0.01).  Four matmul hops,
processed in groups of 4 channels:
  A: pA[w,  (ci,h2)]   = X_c^T @ U^T        (data-stationary, per channel)
  B: pZ_t[w2t, (j,h2)] = U_t @ sP           (matrix-stationary, per w2-chunk)
  f: Lrelu on ScalarE during PSUM->SBUF evac (FD=1024 per instr)
  C: pS[h2m, m*512+j*128+w'] = sA_jmt^T @ D_t^T  (data-stationary, banded)
  D: pY[h', (j,w')]    = D_m^T-chunks @ sS  (matrix-stationary, N=512)
Engines: ScalarE does the two activations + D-evac; VectorE does A/C evacs.
"""
import numpy as np
import ml_dtypes
from contextlib import ExitStack

import concourse.bacc as bacc
import concourse.tile as tile
from concourse import mybir
from concourse.bass_utils import run_bass_kernel_spmd

F32 = mybir.dt.float32
BF16 = mybir.dt.bfloat16
AF = mybir.ActivationFunctionType

N_CORES = 8
B_CORE = 2          # images per core
H = W = C = 128
NEG_SLOPE = 0.01


def _keys_cubic(x):
    x = np.abs(x)
    return np.where(
        x <= 1, (1.5 * x - 2.5) * x * x + 1,
        np.where(x < 2, ((-0.5 * x + 2.5) * x - 4) * x + 2, 0.0))


def _resize_matrix(n_in, n_out):
    """Row-stochastic bicubic (antialias) resize operator; matches
    jax.image.resize(method='bicubic', antialias=True)."""
    scale = n_out / n_in
    pos = (np.arange(n_out) + 0.5) / scale - 0.5
    kscale = min(scale, 1.0)
    w = _keys_cubic((np.arange(n_in)[None, :] - pos[:, None]) * kscale)
    return (w / w.sum(axis=1, keepdims=True)).astype(np.float64)


def _band(Dm, t):
    rows = np.nonzero(np.abs(Dm[:, t * 128:(t + 1) * 128]).sum(1) > 0)[0]
    return int(rows.min()), int(rows.max()) + 1


_CACHE = {}


def _build(repeat=1):
    if repeat in _CACHE:
        return _CACHE[repeat]

    U = _resize_matrix(H, 2 * H)       # [256,128]
    Dm = _resize_matrix(2 * H, H)      # [128,256]
    uT = U.T.astype(ml_dtypes.bfloat16)                              # [128,256]
    dT = np.concatenate([Dm.T[0:128, :], Dm.T[128:256, :]], axis=1)  # [128,256]
    dT_bf = dT.astype(ml_dtypes.bfloat16)
    bands = [_band(Dm, 0), _band(Dm, 1)]   # [(0,66),(62,128)]

    nc = bacc.Bacc()
    x_d = nc.declare_dram_parameter("x", [B_CORE, H, W, C], BF16, isOutput=False)
    ut_d = nc.declare_dram_parameter("ut", [128, 256], BF16, isOutput=False)
    dbf_d = nc.declare_dram_parameter("dbf", [128, 256], BF16, isOutput=False)
    y_d = nc.declare_dram_parameter("y", [B_CORE, H, W, C], BF16, isOutput=True)

    with tile.TileContext(nc) as tc, ExitStack() as ctx:
        wpool = ctx.enter_context(tc.tile_pool(name="weights", bufs=1))
        xpool = ctx.enter_context(tc.tile_pool(name="ximg", bufs=2))
        opool = ctx.enter_context(tc.tile_pool(name="oimg", bufs=2))
        spool = ctx.enter_context(tc.tile_pool(name="stage", bufs=2))
        ppool = ctx.enter_context(tc.tile_pool(name="psum", bufs=1, space="PSUM"))

        ut_s = wpool.tile([128, 256], BF16, tag="ut")
        dbf_s = wpool.tile([128, 256], BF16, tag="dbf")
        nc.sync.dma_start(ut_s[:], ut_d[:])
        nc.sync.dma_start(dbf_s[:], dbf_d[:])

        for b in [ib for _ in range(repeat) for ib in range(B_CORE)]:
            ximg = xpool.tile([128, W * C], BF16, tag="ximg")
            nc.sync.dma_start(ximg[:], x_d[b].rearrange("h w c -> h (w c)"))
            oimg = opool.tile([128, W * C], BF16, tag="oimg")

            for g in range(C // 4):          # 4-channel groups
                c0 = g * 4
                # ---- A: per pair p, pA_p[w, (ci,h2)] = X_c^T @ U^T
                sP = spool.tile([128, 1024], BF16, tag="sP")
                for p in range(2):
                    pA = ppool.tile([128, 512], F32, tag="pA", bufs=2)
                    for ci in range(2):
                        nc.tensor.matmul(pA[:, ci * 256:(ci + 1) * 256],
                                         ximg[:, (c0 + 2 * p + ci)::C], ut_s[:],
                                         start=True, stop=True)
                    nc.vector.tensor_copy(sP[:, p * 512:(p + 1) * 512], pA[:])

                # ---- B: pZ_t[w2t, (j,h2)] = U_t @ sP ; lrelu -> sA
                sA = spool.tile([128, 2048], BF16, tag="sA")
                for t in range(2):
                    pZ = ppool.tile([128, 1024], F32, tag="pZ", bufs=1)
                    for p in range(2):
                        nc.tensor.matmul(pZ[:, p * 512:(p + 1) * 512],
                                         ut_s[:, t * 128:(t + 1) * 128],
                                         sP[:, p * 512:(p + 1) * 512],
                                         start=True, stop=True)
                    nc.scalar.activation(sA[:, t * 1024:(t + 1) * 1024],
                                         pZ[:], AF.Lrelu, alpha=NEG_SLOPE)

                # ---- C: banded W-down; pS cols = m*512 + j*128 + w'
                pS = ppool.tile([128, 1024], F32, tag="pS", bufs=1)
                for j in range(4):
                    for m in range(2):
                        for t in range(2):
                            lo, hi = bands[t]
                            nc.tensor.matmul(
                                pS[:, m * 512 + j * 128 + lo:
                                   m * 512 + j * 128 + hi],
                                sA[:, t * 1024 + j * 256 + m * 128:
                                   t * 1024 + j * 256 + (m + 1) * 128],
                                dbf_s[:, t * 128 + lo:t * 128 + hi],
                                start=(t == 0), stop=(t == 1),
                                skip_group_check=True)
                sS = spool.tile([128, 1024], BF16, tag="sS")
                nc.vector.tensor_copy(sS[:], pS[:])

                # ---- D: pY[h', (j,w')] = sum_m D-chunk_m @ sS[:, m*512:]
                pY = ppool.tile([128, 512], F32, tag="pY", bufs=2)
                for m in range(2):
                    nc.tensor.matmul(pY[:],
                                     dbf_s[:, m * 128:(m + 1) * 128],
                                     sS[:, m * 512:(m + 1) * 512],
                                     start=(m == 0), stop=(m == 1))
                # ---- evac pY (j,w') -> oimg cols w'*C + c0 + j
                dsto = oimg[:].rearrange(
                    "h (w c) -> h w c", c=C)[:, :, c0:c0 + 4]
                srco = pY[:].rearrange("h (c w) -> h w c", c=4)
                nc.scalar.copy(dsto, srco)

            nc.sync.dma_start(y_d[b].rearrange("h w c -> h (w c)"), oimg[:])

    nc.compile()
    consts = {"ut": np.ascontiguousarray(uT),
              "dbf": np.ascontiguousarray(dT_bf)}
    _CACHE[repeat] = (nc, consts)
    return nc, consts


def prepare(x, repeat=1):
    x = np.asarray(x, dtype=np.float32)
    assert x.shape == (16, H, W, C), x.shape
    nc, consts = _build(repeat)
    in_maps = []
    for core in range(N_CORES):
        m = {"x": np.ascontiguousarray(
            x[core * B_CORE:(core + 1) * B_CORE]).astype(ml_dtypes.bfloat16)}
        m.update(consts)
        in_maps.append(m)
    return nc, in_maps


def kernel(x, in_size=128, out_size=128, trace=False, tmpdir=None):
    nc, in_maps = prepare(x)
    res = run_bass_kernel_spmd(nc, in_maps, list(range(N_CORES)), trace=trace,
                               tmpdir=tmpdir)
    out = np.concatenate([res.results[i]["y"] for i in range(N_CORES)], axis=0)
    if trace:
        kernel.last_exec_time_ns = res.exec_time_ns
        kernel.last_results = res
    return out.astype(np.float32)
